# revision 1
# baseline (speedup 1.0000x reference)
"""Trainium2 Bass kernel for GQA attention prefill (B=1, S=2048, D=4096,
32 Q heads / 8 KV heads, HD=128, RoPE, causal-masked softmax, output proj).

Sharding: tensor-parallel over heads across 8 NeuronCores. Core c computes
Q heads 4c..4c+3 with KV head c, then its partial contribution
attn_heads_c @ wo[rows of those heads]; the host sums the 8 partials
(the "all-reduce" after wo).

All matmuls run as float32r (TF32-like reduced-precision fp32: full PE rate
at free-dim>=256, ~1e-4 relative error). Everything on-chip is kept in
"transposed" layout [feature, seq] so that Q/K projections, scores, PV and
the wo matmul all contract along the partition dim with no transposes,
except V which is transposed to [seq, hd] via 4 PE-transposes per 512 rows.

RoPE: wq/wk columns are permuted on the host so rotary pairs (2i, 2i+1)
land on partitions (i, i+64); RoPE is then 2 half-partition copies + 3
full-width DVE ops per [128, 512] tile against host-prepared cos/sin tables.

Softmax: scores are computed transposed [k, q]; no max-subtraction (scores
are O(10) for this problem; exp is safe in fp32). Sum over k (= partition
dim) rides on an extra all-ones [128,128] matmul accumulated alongside PV,
which also broadcasts the sums to all partitions for the final normalize.

Mask: the host inspects the mask input. All-zeros -> no masking. Exact
causal triu(-1e9) -> upper-triangle k-tiles are skipped entirely and only
the 4 diagonal-crossing patterns (precomputed host-side) are added.
Anything else -> generic additive mask streamed from DRAM (pre-scaled by
sqrt(HD) so the fused exp(scale*(s+m')) equals exp(scale*s + m)).
"""

import os
import sys
import types
from contextlib import ExitStack

import numpy as np

for _p in ("/opt/trn_rl_repo",):
    if _p not in sys.path:
        sys.path.insert(0, _p)


def _install_ntff_hook():
    """Best-effort registration of the axon NTFF profiling hook so that
    run_bass_kernel_spmd(trace=True) / BASS_TRACE=1 can report HW exec time.
    Harmless no-op if anything is missing."""
    try:
        import antenv

        if getattr(antenv, "axon_hooks", None) is not None:
            return
        mod = types.ModuleType("antenv.axon_hooks")
        holder = {}
        mod.set_axon_ntff_profile_hook = lambda h: holder.__setitem__("h", h)
        mod.get_axon_ntff_profile_hook = lambda: holder.get("h")
        sys.modules["antenv.axon_hooks"] = mod
        antenv.axon_hooks = mod
        from trn_agent_boot.trn_boot import _ntff_profile_via_ctypes

        h = _ntff_profile_via_ctypes("/opt/axon/libaxon_pjrt.so")
        if h is not None:
            mod.set_axon_ntff_profile_hook(h)
    except Exception:
        pass


_install_ntff_hook()

import concourse.bass as bass  # noqa: E402
import concourse.tile as tile  # noqa: E402
from concourse import bacc, mybir  # noqa: E402
from concourse import bass_utils  # noqa: E402

F32 = mybir.dt.float32
F32R = mybir.dt.float32r
EXP = mybir.ActivationFunctionType.Exp
LN = mybir.ActivationFunctionType.Ln

NCORES = 8
D = 4096
NH, NKV, HD = 32, 8, 128
HPC = NH // NCORES  # 4 query heads per core
SCALE = float(HD) ** -0.5
NEG = -1e9
SB = 512  # seq block (matmul free dim)

_PROG_CACHE: dict = {}
LAST_RESULTS = None  # BassKernelResults of the most recent run (for test.py)


def _build(S: int, mask_mode: str):
    """Emit + compile the per-core Bass program. mask_mode: none|causal|general."""
    NB = S // SB        # seq blocks of 512
    DT = D // 128       # contraction tiles for projections
    KT = S // 128       # k tiles
    DIAG = SB // 128    # k-tiles crossing the diagonal per q block (4)

    nc = bacc.Bacc("TRN2", target_bir_lowering=False, debug=False,
                   num_devices=NCORES)

    def din(name, shape):
        return nc.dram_tensor(name, shape, F32, kind="ExternalInput").ap()

    xt4 = din("xt4", [D // 128, NB, 128, SB])   # pre-tiled x.T [di, b, p, j]
    wq = din("wq", [D, HPC * HD])     # rope-permuted columns
    wk = din("wk", [D, HD])           # rope-permuted columns
    wv = din("wv", [D, HD])
    wo4 = din("wo4", [D // SB, 128, HPC, SB])   # pre-tiled wo [n, p, h, m]
    cos2 = din("cos2", [128, S])      # rows j and j+64 = cos(ang[:, j])
    sin2 = din("sin2", [128, S])      # row j = -sin, row j+64 = +sin
    ident = din("ident", [128, 128])
    ones = din("ones", [128, 128])
    if mask_mode == "causal":
        pats = din("pats", [DIAG * 128, SB])
    if mask_mode == "general":
        maskt4 = din("maskt4", [NB, 128, KT, SB])  # pre-tiled mask.T*sqrt(HD)
    out4 = nc.dram_tensor("o4", [S // 128, D // SB, 128, SB], F32,
                          kind="ExternalOutput").ap()

    with tile.TileContext(nc) as tc, ExitStack() as ctx:
        # ---- persistent activations (live through all phases) ----
        apool = ctx.enter_context(tc.tile_pool(name="acts", bufs=1))
        xq_sb = apool.tile([128, HPC * S], F32R, tag="xq")  # per-head [hd, s]
        xk_sb = apool.tile([128, S], F32R, tag="xk")
        v_sb = apool.tile([128, S], F32R, tag="v")          # [s%128, hd] tiles

        # ================= Phase A: projections + RoPE + V transpose ======
        with tc.tile_pool(name="wproj", bufs=1) as wpool, \
             tc.tile_pool(name="aconst", bufs=1) as acpool, \
             tc.tile_pool(name="xin", bufs=8) as xpool, \
             tc.tile_pool(name="ptmp", bufs=2) as tpool, \
             tc.tile_pool(name="pjps", bufs=1, space="PSUM") as pjps, \
             tc.tile_pool(name="vtps", bufs=1, space="PSUM") as vtps:
            wq_sb = wpool.tile([128, DT * HPC * HD], F32R, tag="wq")
            wk_sb = wpool.tile([128, DT * HD], F32R, tag="wk")
            wv_sb = wpool.tile([128, DT * HD], F32R, tag="wv")
            cos_sb = acpool.tile([128, S], F32, tag="cos")
            sin_sb = acpool.tile([128, S], F32, tag="sin")
            id_sb = acpool.tile([128, 128], F32R, tag="id")
            nc.scalar.dma_start(cos_sb[:], cos2)
            nc.scalar.dma_start(sin_sb[:], sin2)
            nc.scalar.dma_start(id_sb[:], ident.bitcast(F32R))

            def rope(ps, dst, b):
                cs = cos_sb[:, b * SB:(b + 1) * SB]
                sn = sin_sb[:, b * SB:(b + 1) * SB]
                t2 = tpool.tile([128, SB], F32, tag="t2")
                nc.vector.tensor_mul(t2[:], ps[:], cs)
                swp = tpool.tile([128, SB], F32, tag="swp")
                nc.scalar.copy(swp[0:64, :], ps[64:128, :])
                nc.scalar.copy(swp[64:128, :], ps[0:64, :])
                t1 = tpool.tile([128, SB], F32, tag="t1")
                nc.vector.tensor_mul(t1[:], swp[:], sn)
                nc.vector.tensor_add(dst, t1[:], t2[:])

            for b in range(NB):
                pq = [pjps.tile([128, SB], F32, tag=f"pq{h}", name=f"pq{h}",
                                bufs=2 if h == 0 else 1)
                      for h in range(HPC)]
                pk = pjps.tile([128, SB], F32, tag="pk")
                pv = pjps.tile([128, SB], F32, tag="pv")
                for di in range(DT):
                    if b == 0:
                        if di == 0:
                            for hh in range(HPC):
                                nc.sync.dma_start(
                                    wq_sb[:, hh * HD:(hh + 1) * HD],
                                    wq[0:128, hh * HD:(hh + 1) * HD].bitcast(F32R))
                        else:
                            nc.sync.dma_start(
                                wq_sb[:, di * HPC * HD:(di + 1) * HPC * HD],
                                wq[di * 128:(di + 1) * 128, :].bitcast(F32R))
                        nc.sync.dma_start(
                            wk_sb[:, di * HD:(di + 1) * HD],
                            wk[di * 128:(di + 1) * 128, :].bitcast(F32R))
                        nc.sync.dma_start(
                            wv_sb[:, di * HD:(di + 1) * HD],
                            wv[di * 128:(di + 1) * 128, :].bitcast(F32R))
                    xt_t = xpool.tile([128, SB], F32R, tag="xt")
                    nc.gpsimd.dma_start(xt_t[:], xt4[di, b].bitcast(F32R))
                    st, sp = (di == 0), (di == DT - 1)
                    for h in range(HPC):
                        nc.tensor.matmul(
                            pq[h][:],
                            wq_sb[:, di * HPC * HD + h * HD: di * HPC * HD + (h + 1) * HD],
                            xt_t[:], start=st, stop=sp)
                    nc.tensor.matmul(pk[:], wk_sb[:, di * HD:(di + 1) * HD],
                                     xt_t[:], start=st, stop=sp)
                    nc.tensor.matmul(pv[:], wv_sb[:, di * HD:(di + 1) * HD],
                                     xt_t[:], start=st, stop=sp)
                for h in range(HPC):
                    rope(pq[h], xq_sb[:, h * S + b * SB: h * S + (b + 1) * SB], b)
                rope(pk, xk_sb[:, b * SB:(b + 1) * SB], b)
                # V: [hd, s] psum -> natural [s, hd] via 4 PE transposes
                vt = tpool.tile([128, SB], F32R, tag="vt")
                nc.scalar.copy(vt[:], pv[:])
                pvn = vtps.tile([128, SB], F32R, tag="pvn")
                for j in range(SB // 128):
                    nc.tensor.transpose(pvn[:, j * 128:(j + 1) * 128],
                                        vt[:, j * 128:(j + 1) * 128], id_sb[:])
                nc.scalar.copy(v_sb[:, b * SB:(b + 1) * SB], pvn[:])

        # ========== Phases B+C: attention + output proj, pipelined =======
        # Flat software-pipelined loop over attention tiles (h, Q, t):
        # producer P(i) = scores matmul -> mask add (DVE) -> exp (ACT);
        # consumer K(i) = sums matmul + PV matmul, emitted LOOK tiles later
        # so the exp latency stays off PE's critical path. The wo-projection
        # matmul chunks for q-block Q-1 are interleaved as PE filler, which
        # keeps PE busy while ACT churns exps (an exp of a [128,512] tile
        # costs ~2-3 matmuls of PE time). Group tails compute 1/sum as
        # exp(-ln(sum)) on ACT, keeping the slow DVE reciprocal off the
        # PSUM-release path.
        LOOK = 3
        at_pool = ctx.enter_context(tc.tile_pool(name="attn", bufs=1))
        at_sb = at_pool.tile([128, HPC * S], F32R, tag="at")
        with tc.tile_pool(name="bconst", bufs=1) as bcpool, \
             tc.tile_pool(name="probs", bufs=8) as ppool, \
             tc.tile_pool(name="btmp", bufs=3) as btpool, \
             tc.tile_pool(name="woc", bufs=2) as wop, \
             tc.tile_pool(name="osb", bufs=4) as opool, \
             tc.tile_pool(name="sps", bufs=3, space="PSUM") as sps, \
             tc.tile_pool(name="atps", bufs=2, space="PSUM") as atps, \
             tc.tile_pool(name="smps", bufs=1, space="PSUM") as smps, \
             tc.tile_pool(name="ops", bufs=2, space="PSUM") as ops, \
             ExitStack() as bctx:
            ones_sb = bcpool.tile([128, 128], F32R, tag="ones")
            nc.sync.dma_start(ones_sb[:], ones.bitcast(F32R))
            if mask_mode == "causal":
                pats_sb = bcpool.tile([128, DIAG * SB], F32, tag="pats")
                for m in range(DIAG):
                    nc.sync.dma_start(pats_sb[:, m * SB:(m + 1) * SB],
                                      pats[m * 128:(m + 1) * 128, :])
            if mask_mode == "general":
                mpool = bctx.enter_context(tc.tile_pool(name="mstrip", bufs=1))

            def wo_fill(Qc):
                """Generator of phase-C chunk emitters for q-block Qc."""
                for n in range(D // SB):
                    woc = wop.tile([128, HPC * SB], F32R, tag="woc", name="woc")
                    nc.gpsimd.dma_start(
                        woc[:].rearrange("p (h m) -> p h m", m=SB),
                        wo4[n].bitcast(F32R))
                    for s in range(Qc * DIAG, (Qc + 1) * DIAG):
                        def chunk(n=n, s=s, woc=woc):
                            po = ops.tile([128, SB], F32, tag="po", name="po")
                            for h in range(HPC):
                                nc.tensor.matmul(
                                    po[:],
                                    at_sb[:, h * S + s * 128: h * S + (s + 1) * 128],
                                    woc[:, h * SB:(h + 1) * SB],
                                    start=(h == 0), stop=(h == HPC - 1))
                            ot = opool.tile([128, SB], F32, tag="ot", name="ot")
                            nc.vector.tensor_copy(ot[:], po[:])
                            nc.sync.dma_start(out4[s, n], ot[:])
                        yield chunk

            state = {}  # i -> (pr, pat_, psm, h, Q, t, kmax)

            def produce(i, h, Q, t, kmax, mstrip):
                qs = xq_sb[:, h * S + Q * SB: h * S + (Q + 1) * SB]
                pss = sps.tile([128, SB], F32, tag="pss", name="pss")
                nc.tensor.matmul(pss[:], xk_sb[:, t * 128:(t + 1) * 128],
                                 qs, start=True, stop=True)
                if mask_mode == "causal" and t >= kmax - DIAG:
                    m = t - (kmax - DIAG)
                    nc.vector.tensor_add(pss[:], pss[:],
                                         pats_sb[:, m * SB:(m + 1) * SB])
                elif mask_mode == "general":
                    nc.vector.tensor_add(pss[:], pss[:],
                                         mstrip[:, t * SB:(t + 1) * SB])
                pr = ppool.tile([128, SB], F32R, tag="pr", name="pr")
                nc.scalar.activation(pr[:], pss[:], EXP, scale=SCALE)
                if t == 0:
                    pat_ = atps.tile([128, SB], F32, tag="pat", name="pat")
                    psm = smps.tile([128, SB], F32, tag="psm", name="psm")
                else:
                    _, pat_, psm = state[i - 1][:3]
                state[i] = (pr, pat_, psm, h, Q, t, kmax)

            def consume(i):
                pr, pat_, psm, h, Q, t, kmax = state.pop(i)
                nc.tensor.matmul(psm[:], ones_sb[:], pr[:],
                                 start=(t == 0), stop=(t == kmax - 1))
                nc.tensor.matmul(pat_[:], v_sb[:, t * 128:(t + 1) * 128],
                                 pr[:], start=(t == 0), stop=(t == kmax - 1))
                if t == kmax - 1:
                    # 1/s as exp(-ln(s)) on ACT: ~2x690ns, frees the DVE
                    # queue and releases the psm bank immediately after ln.
                    lns = btpool.tile([128, SB], F32, tag="lns", name="lns")
                    nc.scalar.activation(lns[:], psm[:], LN)
                    rcp = btpool.tile([128, SB], F32, tag="rcp", name="rcp")
                    nc.scalar.activation(rcp[:], lns[:], EXP, scale=-1.0)
                    nc.vector.tensor_mul(
                        at_sb[:, h * S + Q * SB: h * S + (Q + 1) * SB],
                        pat_[:], rcp[:])

            for Q in range(NB):
                kmax = DIAG * (Q + 1) if mask_mode == "causal" else KT
                mstrip = None
                if mask_mode == "general":
                    mstrip = mpool.tile([128, KT * SB], F32, tag="ms", name="ms")
                    nc.sync.dma_start(
                        mstrip[:].rearrange("p (t j) -> p t j", j=SB),
                        maskt4[Q])
                tiles = [(h, t) for h in range(HPC) for t in range(kmax)]
                fillers = list(wo_fill(Q - 1)) if Q > 0 else []
                nf, nt = len(fillers), len(tiles)
                fdone = 0
                base = Q * 10000
                for i, (h, t) in enumerate(tiles):
                    produce(base + i, h, Q, t, kmax, mstrip)
                    while fdone * nt < nf * (i + 1):
                        fillers[fdone]()
                        fdone += 1
                    if i >= LOOK:
                        consume(base + i - LOOK)
                for i in range(nt - LOOK, nt):
                    consume(base + i)
                while fdone < nf:
                    fillers[fdone]()
                    fdone += 1
            for f in wo_fill(NB - 1):
                f()

    nc.compile()
    return nc


def _get_prog(S: int, mask_mode: str):
    key = (S, mask_mode)
    if key not in _PROG_CACHE:
        _PROG_CACHE[key] = _build(S, mask_mode)
    return _PROG_CACHE[key]


def _mask_mode(mask: np.ndarray) -> str:
    S = mask.shape[0]
    if not mask.any():
        return "none"
    causal = np.triu(np.full((S, S), np.float32(NEG), dtype=np.float32), k=1)
    if np.array_equal(mask, causal):
        return "causal"
    return "general"


def kernel(x, wq, wk, wv, wo, freqs_cos, freqs_sin, positions, mask):
    x = np.asarray(x, dtype=np.float32)
    B = x.shape[0]
    assert B == 1
    S = x.shape[1]
    x2 = np.ascontiguousarray(x[0])                 # [S, D]
    mask = np.asarray(mask, dtype=np.float32)
    mode = _mask_mode(mask)
    nc = _get_prog(S, mode)

    xt = x2.T                                        # [D, S]
    DT, NB = D // 128, S // SB
    xt4 = np.ascontiguousarray(
        xt.reshape(DT, 128, NB, SB).transpose(0, 2, 1, 3))
    perm = np.concatenate([np.arange(0, HD, 2), np.arange(1, HD, 2)])
    cosT = np.ascontiguousarray(np.asarray(freqs_cos, np.float32).T)  # [64, S]
    sinT = np.ascontiguousarray(np.asarray(freqs_sin, np.float32).T)
    cos2 = np.concatenate([cosT, cosT], axis=0)     # [128, S]
    sin2 = np.concatenate([-sinT, sinT], axis=0)
    ident = np.eye(128, dtype=np.float32)
    ones = np.ones((128, 128), dtype=np.float32)

    common = {"xt4": xt4, "cos2": cos2, "sin2": sin2, "ident": ident,
              "ones": ones}
    if mode == "causal":
        DIAG = SB // 128
        i = np.arange(128)[:, None]
        j = np.arange(SB)[None, :]
        pats = np.concatenate(
            [np.where(128 * m + i > j, np.float32(NEG), np.float32(0.0))
             for m in range(DIAG)], axis=0).astype(np.float32)
        common["pats"] = pats
    if mode == "general":
        KT = S // 128
        mt = (mask.T * np.float32(np.sqrt(HD))).astype(np.float32)
        common["maskt4"] = np.ascontiguousarray(
            mt.reshape(KT, 128, NB, SB).transpose(2, 1, 0, 3))

    wq = np.asarray(wq, np.float32)
    wk = np.asarray(wk, np.float32)
    wv = np.asarray(wv, np.float32)
    wo = np.asarray(wo, np.float32)
    in_maps = []
    for c in range(NCORES):
        hs = slice(c * HPC * HD, (c + 1) * HPC * HD)
        wq_c = wq[:, hs].reshape(D, HPC, HD)[:, :, perm].reshape(D, HPC * HD)
        wk_c = wk[:, c * HD:(c + 1) * HD][:, perm]
        wo_c = wo[hs, :]
        wo4 = np.ascontiguousarray(
            wo_c.reshape(HPC, 128, D // SB, SB).transpose(2, 1, 0, 3))
        in_maps.append(dict(
            common,
            wq=np.ascontiguousarray(wq_c),
            wk=np.ascontiguousarray(wk_c),
            wv=np.ascontiguousarray(wv[:, c * HD:(c + 1) * HD]),
            wo4=wo4,
        ))

    global LAST_RESULTS
    trace = bool(os.environ.get("BASS_TRACE"))
    res = bass_utils.run_bass_kernel_spmd(
        nc, in_maps, core_ids=list(range(NCORES)), trace=trace)
    LAST_RESULTS = res
    acc = res.results[0]["o4"].astype(np.float32).copy()
    for c in range(1, NCORES):
        acc += res.results[c]["o4"]
    return acc.transpose(0, 2, 1, 3).reshape(1, S, D)



# revision 2
# speedup vs baseline: 1.1576x; 1.1576x over previous
"""Trainium2 Bass kernel for GQA attention prefill (B=1, S=2048, D=4096,
32 Q heads / 8 KV heads, HD=128, RoPE, causal-masked softmax, output proj).

Sharding: tensor-parallel over heads across 8 NeuronCores. Core c computes
Q heads 4c..4c+3 with KV head c, then its partial contribution
attn_heads_c @ wo[rows of those heads]; the host sums the 8 partials
(the "all-reduce" after wo).

All matmul operands are bf16 (PSUM accumulation stays fp32; measured
end-to-end rel err ~5e-3 vs the 2e-2 gate). bf16 matters beyond bandwidth:
fp32r matmuls must self-load PE weights (~90ns stall per matmul measured),
while bf16 LDWEIGHTS are split off and overlap the previous matmul's
stream, keeping the PE array continuously busy (TRN2 PE clock ramps 0.65
-> 2.4 GHz only under continuous execution).

Everything on-chip is kept in "transposed" layout [feature, seq] so that
Q/K projections, scores, PV and the wo matmul all contract along the
partition dim with no transposes, except V which is transposed to
[seq, hd] via 4 PE-transposes per 512 rows.

RoPE: wq/wk columns are permuted on the host so rotary pairs (2i, 2i+1)
land on partitions (i, i+64); RoPE is then 2 half-partition copies + 3
full-width DVE ops per [128, 512] tile against host-prepared cos/sin
tables.

Softmax: scores are computed transposed [k, q]; no max-subtraction (scores
are O(10) for this problem; exp is safe in fp32). Sum over k (= partition
dim) rides on an extra all-ones [128,128] matmul accumulated alongside PV,
which also broadcasts the sums to all partitions. 1/sum runs on the DVE
(reciprocal_approx_fast) — computing it on ACT (ln+exp) forced ~1.3us
activation-table reloads 2x per softmax group.

Mask: the host inspects the mask input. All-zeros -> no masking. Exact
causal triu(-1e9) -> upper-triangle k-tiles are skipped entirely and only
the 4 diagonal-crossing patterns (precomputed host-side) are added.
Anything else -> generic additive mask streamed from DRAM (pre-scaled by
sqrt(HD) so the fused exp(scale*(s+m')) equals exp(scale*s + m)).

Output partials are written as bf16 (halves the output DMA and the
psum->sbuf copy cost; the copies alternate between the Scalar and Vector
engines so neither becomes the bottleneck); the host accumulates in fp32.
"""

import os
import sys
import types
from contextlib import ExitStack

import numpy as np
import ml_dtypes

BF16NP = ml_dtypes.bfloat16

for _p in ("/opt/trn_rl_repo",):
    if _p not in sys.path:
        sys.path.insert(0, _p)


def _install_ntff_hook():
    """Best-effort registration of the axon NTFF profiling hook so that
    run_bass_kernel_spmd(trace=True) / BASS_TRACE=1 can report HW exec time.
    Harmless no-op if anything is missing."""
    try:
        import antenv

        if getattr(antenv, "axon_hooks", None) is not None:
            return
        mod = types.ModuleType("antenv.axon_hooks")
        holder = {}
        mod.set_axon_ntff_profile_hook = lambda h: holder.__setitem__("h", h)
        mod.get_axon_ntff_profile_hook = lambda: holder.get("h")
        sys.modules["antenv.axon_hooks"] = mod
        antenv.axon_hooks = mod
        from trn_agent_boot.trn_boot import _ntff_profile_via_ctypes

        h = _ntff_profile_via_ctypes("/opt/axon/libaxon_pjrt.so")
        if h is not None:
            mod.set_axon_ntff_profile_hook(h)
    except Exception:
        pass


_install_ntff_hook()

import concourse.bass as bass  # noqa: E402
import concourse.tile as tile  # noqa: E402
from concourse import bacc, mybir  # noqa: E402
from concourse import bass_utils  # noqa: E402

F32 = mybir.dt.float32
F32R = mybir.dt.float32r
BF16 = mybir.dt.bfloat16
EXP = mybir.ActivationFunctionType.Exp

NCORES = 8
D = 4096
NH, NKV, HD = 32, 8, 128
HPC = NH // NCORES  # 4 query heads per core
SCALE = float(HD) ** -0.5
NEG = -1e9
SB = 512  # seq block (matmul free dim)

_PROG_CACHE: dict = {}
LAST_RESULTS = None  # BassKernelResults of the most recent run (for test.py)


def _build(S: int, mask_mode: str):
    """Emit + compile the per-core Bass program. mask_mode: none|causal|general."""
    NB = S // SB        # seq blocks of 512
    DT = D // 128       # contraction tiles for projections
    KT = S // 128       # k tiles
    DIAG = SB // 128    # k-tiles crossing the diagonal per q block (4)

    nc = bacc.Bacc("TRN2", target_bir_lowering=False, debug=False,
                   num_devices=NCORES)

    def din(name, shape, dt=BF16):
        return nc.dram_tensor(name, shape, dt, kind="ExternalInput").ap()

    xt4 = din("xt4", [D // 128, NB, 128, SB])   # pre-tiled x.T [di, b, p, j]
    wq = din("wq", [D, HPC * HD])     # rope-permuted columns
    wk = din("wk", [D, HD])           # rope-permuted columns
    wv = din("wv", [D, HD])
    wo4 = din("wo4", [D // SB, 128, HPC, SB])   # pre-tiled wo [n, p, h, m]
    cos2 = din("cos2", [128, S], F32)  # rows j and j+64 = cos(ang[:, j])
    sin2 = din("sin2", [128, S], F32)  # row j = -sin, row j+64 = +sin
    ident = din("ident", [128, 128], F32)
    ones = din("ones", [128, 128])
    if mask_mode == "causal":
        pats = din("pats", [DIAG * 128, SB], F32)
    if mask_mode == "general":
        maskt4 = din("maskt4", [NB, 128, KT, SB], F32)  # pre-tiled mask.T*sqrt(HD)
    out4 = nc.dram_tensor("o4", [S // 128, D // SB, 128, SB], BF16,
                          kind="ExternalOutput").ap()

    with tile.TileContext(nc) as tc, ExitStack() as ctx:
        # ---- persistent activations (live through all phases) ----
        apool = ctx.enter_context(tc.tile_pool(name="acts", bufs=1))
        xq_sb = apool.tile([128, HPC * S], BF16, tag="xq")  # per-head [hd, s]
        xk_sb = apool.tile([128, S], BF16, tag="xk")
        v_sb = apool.tile([128, S], BF16, tag="v")          # [s%128, hd] tiles

        # phase-B constants, loaded up front so the A->B seam has no DMA wait
        bcpool = ctx.enter_context(tc.tile_pool(name="bconst", bufs=1))
        ones_sb = bcpool.tile([128, 128], BF16, tag="ones")
        nc.sync.dma_start(ones_sb[:], ones)
        if mask_mode == "causal":
            pats_sb = bcpool.tile([128, DIAG * SB], F32, tag="pats")
            for m in range(DIAG):
                nc.sync.dma_start(pats_sb[:, m * SB:(m + 1) * SB],
                                  pats[m * 128:(m + 1) * 128, :])

        # ================= Phase A: projections + RoPE + V transpose ======
        with tc.tile_pool(name="wproj", bufs=1) as wpool, \
             tc.tile_pool(name="aconst", bufs=1) as acpool, \
             tc.tile_pool(name="xin", bufs=8) as xpool, \
             tc.tile_pool(name="ptmp", bufs=2) as tpool, \
             tc.tile_pool(name="pjps", bufs=1, space="PSUM") as pjps, \
             tc.tile_pool(name="vtps", bufs=1, space="PSUM") as vtps:
            wq_sb = wpool.tile([128, DT * HPC * HD], BF16, tag="wq")
            wk_sb = wpool.tile([128, DT * HD], BF16, tag="wk")
            wv_sb = wpool.tile([128, DT * HD], BF16, tag="wv")
            cos_sb = acpool.tile([128, S], F32, tag="cos")
            sin_sb = acpool.tile([128, S], F32, tag="sin")
            id_sb = acpool.tile([128, 128], F32R, tag="id")
            nc.scalar.dma_start(cos_sb[:], cos2)
            nc.scalar.dma_start(sin_sb[:], sin2)
            nc.scalar.dma_start(id_sb[:], ident.bitcast(F32R))

            def rope(ps, dst, b):
                cs = cos_sb[:, b * SB:(b + 1) * SB]
                sn = sin_sb[:, b * SB:(b + 1) * SB]
                t2 = tpool.tile([128, SB], F32, tag="t2")
                nc.vector.tensor_mul(t2[:], ps[:], cs)
                swp = tpool.tile([128, SB], F32, tag="swp")
                nc.scalar.copy(swp[0:64, :], ps[64:128, :])
                nc.scalar.copy(swp[64:128, :], ps[0:64, :])
                t1 = tpool.tile([128, SB], F32, tag="t1")
                nc.vector.tensor_mul(t1[:], swp[:], sn)
                nc.vector.tensor_add(dst, t1[:], t2[:])

            for b in range(NB):
                pq = [pjps.tile([128, SB], F32, tag=f"pq{h}", name=f"pq{h}",
                                bufs=2 if h == 0 else 1)
                      for h in range(HPC)]
                pk = pjps.tile([128, SB], F32, tag="pk")
                pv = pjps.tile([128, SB], F32, tag="pv")
                for di in range(DT):
                    if b == 0:
                        if di == 0:
                            for hh in range(HPC):
                                nc.sync.dma_start(
                                    wq_sb[:, hh * HD:(hh + 1) * HD],
                                    wq[0:128, hh * HD:(hh + 1) * HD])
                        else:
                            nc.sync.dma_start(
                                wq_sb[:, di * HPC * HD:(di + 1) * HPC * HD],
                                wq[di * 128:(di + 1) * 128, :])
                        nc.sync.dma_start(
                            wk_sb[:, di * HD:(di + 1) * HD],
                            wk[di * 128:(di + 1) * 128, :])
                        nc.sync.dma_start(
                            wv_sb[:, di * HD:(di + 1) * HD],
                            wv[di * 128:(di + 1) * 128, :])
                    xt_t = xpool.tile([128, SB], BF16, tag="xt")
                    nc.gpsimd.dma_start(xt_t[:], xt4[di, b])
                    st, sp = (di == 0), (di == DT - 1)
                    for h in range(HPC):
                        nc.tensor.matmul(
                            pq[h][:],
                            wq_sb[:, di * HPC * HD + h * HD: di * HPC * HD + (h + 1) * HD],
                            xt_t[:], start=st, stop=sp)
                    nc.tensor.matmul(pk[:], wk_sb[:, di * HD:(di + 1) * HD],
                                     xt_t[:], start=st, stop=sp)
                    nc.tensor.matmul(pv[:], wv_sb[:, di * HD:(di + 1) * HD],
                                     xt_t[:], start=st, stop=sp)
                for h in range(HPC):
                    rope(pq[h], xq_sb[:, h * S + b * SB: h * S + (b + 1) * SB], b)
                rope(pk, xk_sb[:, b * SB:(b + 1) * SB], b)
                # V: [hd, s] psum -> natural [s, hd] via 4 PE transposes
                vt = tpool.tile([128, SB], F32R, tag="vt")
                nc.scalar.copy(vt[:], pv[:])
                pvn = vtps.tile([128, SB], F32R, tag="pvn")
                for j in range(SB // 128):
                    nc.tensor.transpose(pvn[:, j * 128:(j + 1) * 128],
                                        vt[:, j * 128:(j + 1) * 128], id_sb[:])
                nc.scalar.copy(v_sb[:, b * SB:(b + 1) * SB],
                               pvn[:].bitcast(F32))

        # ========== Phases B+C: attention + output proj, pipelined =======
        # Flat software-pipelined loop over attention tiles (h, Q, t):
        # producer P(i) = scores matmul -> mask add (DVE) -> exp (ACT);
        # consumer K(i) = sums matmul + PV matmul, emitted LOOK tiles later
        # so the exp latency stays off PE's critical path. The wo-projection
        # matmul chunks for q-block Q-1 are interleaved as PE filler, which
        # keeps PE busy while ACT churns exps. Group tails compute 1/sum on
        # the DVE (reciprocal_approx_fast, ~18 bits) off the ACT queue.
        LOOK = 3
        at_pool = ctx.enter_context(tc.tile_pool(name="attn", bufs=1))
        at_sb = at_pool.tile([128, HPC * S], BF16, tag="at")
        with tc.tile_pool(name="probs", bufs=8) as ppool, \
             tc.tile_pool(name="btmp", bufs=3) as btpool, \
             tc.tile_pool(name="woc", bufs=2) as wop, \
             tc.tile_pool(name="osb", bufs=4) as opool, \
             tc.tile_pool(name="sps", bufs=3, space="PSUM") as sps, \
             tc.tile_pool(name="atps", bufs=2, space="PSUM") as atps, \
             tc.tile_pool(name="smps", bufs=1, space="PSUM") as smps, \
             tc.tile_pool(name="ops", bufs=2, space="PSUM") as ops, \
             ExitStack() as bctx:
            if mask_mode == "general":
                mpool = bctx.enter_context(tc.tile_pool(name="mstrip", bufs=1))

            def wo_fill(Qc):
                """Generator of phase-C chunk emitters for q-block Qc."""
                for n in range(D // SB):
                    woc = wop.tile([128, HPC * SB], BF16, tag="woc", name="woc")
                    nc.gpsimd.dma_start(
                        woc[:].rearrange("p (h m) -> p h m", m=SB),
                        wo4[n])
                    for s in range(Qc * DIAG, (Qc + 1) * DIAG):
                        def chunk(n=n, s=s, woc=woc):
                            po = ops.tile([128, SB], F32, tag="po", name="po")
                            for h in range(HPC):
                                nc.tensor.matmul(
                                    po[:],
                                    at_sb[:, h * S + s * 128: h * S + (s + 1) * 128],
                                    woc[:, h * SB:(h + 1) * SB],
                                    start=(h == 0), stop=(h == HPC - 1))
                            ot = opool.tile([128, SB], BF16, tag="ot", name="ot")
                            # alternate psum->sbuf copies across ACT / DVE
                            if (n * DIAG + s) % 2 == 0:
                                nc.scalar.copy(ot[:], po[:])
                            else:
                                nc.vector.tensor_copy(ot[:], po[:])
                            nc.sync.dma_start(out4[s, n], ot[:])
                        yield chunk

            state = {}  # i -> (pr, pat_, psm, h, Q, t, kmax)

            def produce(i, h, Q, t, kmax, mstrip):
                qs = xq_sb[:, h * S + Q * SB: h * S + (Q + 1) * SB]
                pss = sps.tile([128, SB], F32, tag="pss", name="pss")
                nc.tensor.matmul(pss[:], xk_sb[:, t * 128:(t + 1) * 128],
                                 qs, start=True, stop=True)
                if mask_mode == "causal" and t >= kmax - DIAG:
                    m = t - (kmax - DIAG)
                    nc.vector.tensor_add(pss[:], pss[:],
                                         pats_sb[:, m * SB:(m + 1) * SB])
                elif mask_mode == "general":
                    nc.vector.tensor_add(pss[:], pss[:],
                                         mstrip[:, t * SB:(t + 1) * SB])
                pr = ppool.tile([128, SB], BF16, tag="pr", name="pr")
                nc.scalar.activation(pr[:], pss[:], EXP, scale=SCALE)
                if t == 0:
                    pat_ = atps.tile([128, SB], F32, tag="pat", name="pat")
                    psm = smps.tile([128, SB], F32, tag="psm", name="psm")
                else:
                    _, pat_, psm = state[i - 1][:3]
                state[i] = (pr, pat_, psm, h, Q, t, kmax)

            def consume(i):
                pr, pat_, psm, h, Q, t, kmax = state.pop(i)
                nc.tensor.matmul(psm[:], ones_sb[:], pr[:],
                                 start=(t == 0), stop=(t == kmax - 1))
                nc.tensor.matmul(pat_[:], v_sb[:, t * 128:(t + 1) * 128],
                                 pr[:], start=(t == 0), stop=(t == kmax - 1))
                if t == kmax - 1:
                    rcp = btpool.tile([128, SB], F32, tag="rcp", name="rcp")
                    nc.vector.reciprocal_approx_fast(rcp[:], psm[:])
                    nc.vector.tensor_mul(
                        at_sb[:, h * S + Q * SB: h * S + (Q + 1) * SB],
                        pat_[:], rcp[:])

            for Q in range(NB):
                kmax = DIAG * (Q + 1) if mask_mode == "causal" else KT
                mstrip = None
                if mask_mode == "general":
                    mstrip = mpool.tile([128, KT * SB], F32, tag="ms", name="ms")
                    nc.sync.dma_start(
                        mstrip[:].rearrange("p (t j) -> p t j", j=SB),
                        maskt4[Q])
                tiles = [(h, t) for h in range(HPC) for t in range(kmax)]
                fillers = list(wo_fill(Q - 1)) if Q > 0 else []
                nf, nt = len(fillers), len(tiles)
                fdone = 0
                base = Q * 10000
                for i, (h, t) in enumerate(tiles):
                    produce(base + i, h, Q, t, kmax, mstrip)
                    while fdone * nt < nf * (i + 1):
                        fillers[fdone]()
                        fdone += 1
                    if i >= LOOK:
                        consume(base + i - LOOK)
                for i in range(nt - LOOK, nt):
                    consume(base + i)
                while fdone < nf:
                    fillers[fdone]()
                    fdone += 1
            for f in wo_fill(NB - 1):
                f()

    nc.compile()
    return nc


def _get_prog(S: int, mask_mode: str):
    key = (S, mask_mode)
    if key not in _PROG_CACHE:
        _PROG_CACHE[key] = _build(S, mask_mode)
    return _PROG_CACHE[key]


def _mask_mode(mask: np.ndarray) -> str:
    S = mask.shape[0]
    if not mask.any():
        return "none"
    causal = np.triu(np.full((S, S), np.float32(NEG), dtype=np.float32), k=1)
    if np.array_equal(mask, causal):
        return "causal"
    return "general"


def kernel(x, wq, wk, wv, wo, freqs_cos, freqs_sin, positions, mask):
    x = np.asarray(x, dtype=np.float32)
    B = x.shape[0]
    assert B == 1
    S = x.shape[1]
    x2 = np.ascontiguousarray(x[0])                 # [S, D]
    mask = np.asarray(mask, dtype=np.float32)
    mode = _mask_mode(mask)
    nc = _get_prog(S, mode)

    xt = x2.T                                        # [D, S]
    DT, NB = D // 128, S // SB
    xt4 = np.ascontiguousarray(
        xt.reshape(DT, 128, NB, SB).transpose(0, 2, 1, 3)).astype(BF16NP)
    perm = np.concatenate([np.arange(0, HD, 2), np.arange(1, HD, 2)])
    cosT = np.ascontiguousarray(np.asarray(freqs_cos, np.float32).T)  # [64, S]
    sinT = np.ascontiguousarray(np.asarray(freqs_sin, np.float32).T)
    cos2 = np.concatenate([cosT, cosT], axis=0)     # [128, S]
    sin2 = np.concatenate([-sinT, sinT], axis=0)
    ident = np.eye(128, dtype=np.float32)
    ones = np.ones((128, 128), dtype=BF16NP)

    common = {"xt4": xt4, "cos2": cos2, "sin2": sin2, "ident": ident,
              "ones": ones}
    if mode == "causal":
        DIAG = SB // 128
        i = np.arange(128)[:, None]
        j = np.arange(SB)[None, :]
        pats = np.concatenate(
            [np.where(128 * m + i > j, np.float32(NEG), np.float32(0.0))
             for m in range(DIAG)], axis=0).astype(np.float32)
        common["pats"] = pats
    if mode == "general":
        KT = S // 128
        mt = (mask.T * np.float32(np.sqrt(HD))).astype(np.float32)
        common["maskt4"] = np.ascontiguousarray(
            mt.reshape(KT, 128, NB, SB).transpose(2, 1, 0, 3))

    wq = np.asarray(wq, np.float32)
    wk = np.asarray(wk, np.float32)
    wv = np.asarray(wv, np.float32)
    wo = np.asarray(wo, np.float32)
    in_maps = []
    for c in range(NCORES):
        hs = slice(c * HPC * HD, (c + 1) * HPC * HD)
        wq_c = wq[:, hs].reshape(D, HPC, HD)[:, :, perm].reshape(D, HPC * HD)
        wk_c = wk[:, c * HD:(c + 1) * HD][:, perm]
        wo_c = wo[hs, :]
        wo4 = np.ascontiguousarray(
            wo_c.reshape(HPC, 128, D // SB, SB).transpose(2, 1, 0, 3)).astype(BF16NP)
        in_maps.append(dict(
            common,
            wq=np.ascontiguousarray(wq_c).astype(BF16NP),
            wk=np.ascontiguousarray(wk_c).astype(BF16NP),
            wv=np.ascontiguousarray(wv[:, c * HD:(c + 1) * HD]).astype(BF16NP),
            wo4=wo4,
        ))

    global LAST_RESULTS
    trace = bool(os.environ.get("BASS_TRACE"))
    res = bass_utils.run_bass_kernel_spmd(
        nc, in_maps, core_ids=list(range(NCORES)), trace=trace)
    LAST_RESULTS = res
    acc = res.results[0]["o4"].astype(np.float32).copy()
    for c in range(1, NCORES):
        acc += res.results[c]["o4"].astype(np.float32)
    return acc.transpose(0, 2, 1, 3).reshape(1, S, D)
